# revision 1
# baseline (speedup 1.0000x reference)
"""Trainium2 Bass kernel for nn_MLPLoraSubspace.

Math: A = sum(alphas_A * controls_A, 0)  (256,)
      Bv = sum(alphas_A.T * controls_B, 1)  (4096,)
      W = A outer Bv  (rank-1)  -> out = (x @ Bv) outer A + bias
      BatchNorm(training stats) then LeakyReLU(0.2).

Because W is rank-1, out[i,j] = A[j]*t[i] + bias[j] with t = x @ Bv.
Batch stats:  mean_j = A_j*mean(t) + bias_j,  var_j = A_j^2*var(t), so
  act[i,j] = lrelu( gamma_j*A_j/sqrt(A_j^2*var_t+eps) * (t[i]-mean_t) + beta_j )
The bias cancels exactly inside the normalization.

Sharding: data-parallel over batch, 8 cores x 2048 rows. Per-core partial
(sum t, sum t^2) is AllReduce'd (2 floats) to form global batch stats.
"""

import sys

for p in ("/opt/trn_rl_repo", "/root/.axon_site/_ro/trn_rl_repo"):
    if p not in sys.path:
        sys.path.insert(0, p)

import numpy as np

from concourse import bacc, bass, mybir, tile
from concourse.bass_utils import run_bass_kernel_spmd

F32 = mybir.dt.float32
N_CORES = 8
B_FULL, DIN, DOUT = 16384, 4096, 256
B_SHARD = B_FULL // N_CORES          # 2048
M_TILES = B_SHARD // 128             # 16
BN_EPS = 1e-5
NEG_SLOPE = 0.2

_CACHE = {}
_ACT_FUNC_OVERRIDE = None  # for sim testing (sim lacks Prelu)


def _act_func():
    return _ACT_FUNC_OVERRIDE or mybir.ActivationFunctionType.Prelu


def _build(with_beta: bool):
    nc = bacc.Bacc(
        "TRN2",
        target_bir_lowering=False,
        debug=False,
        enable_asserts=False,
        num_devices=N_CORES,
    )
    xs = nc.dram_tensor("xs", [B_SHARD, DIN], F32, kind="ExternalInput").ap()
    bv1 = nc.dram_tensor("bv1", [1, DIN], F32, kind="ExternalInput").ap()
    a2b = nc.dram_tensor("a2b", [128, DOUT], F32, kind="ExternalInput").ap()
    gab = nc.dram_tensor("gab", [128, DOUT], F32, kind="ExternalInput").ap()
    if with_beta:
        beb = nc.dram_tensor("beb", [128, DOUT], F32, kind="ExternalInput").ap()
    out = nc.dram_tensor("out", [B_SHARD, DOUT], F32, kind="ExternalOutput").ap()

    with tile.TileContext(nc) as tc:
        with (
            tc.tile_pool(name="xp", bufs=4) as xp,
            tc.tile_pool(name="scr", bufs=2) as scrp,
            tc.tile_pool(name="cst", bufs=1) as cst,
            tc.tile_pool(name="op", bufs=3) as op,
            tc.tile_pool(name="ps", bufs=1, space="PSUM") as ps,
            tc.tile_pool(name="dram", bufs=1, space="DRAM") as dram,
        ):
            # Warm-up collective: absorbs CC-stream/mesh first-call setup
            # cost while phase 1 streams x. Result unused.
            wi = dram.tile([2, 1], F32, tag="wi")
            wo = dram.tile([2 * N_CORES, 1], F32, tag="wo")
            nc.gpsimd.collective_compute(
                "AllGather",
                mybir.AluOpType.bypass,
                replica_groups=[list(range(N_CORES))],
                ins=[wi.opt()],
                outs=[wo.opt()],
            )
            wi2 = dram.tile([2, 1], F32, tag="wi2")
            wo2 = dram.tile([2 * N_CORES, 1], F32, tag="wo2")
            nc.gpsimd.collective_compute(
                "AllGather",
                mybir.AluOpType.bypass,
                replica_groups=[list(range(N_CORES))],
                ins=[wi2.opt()],
                outs=[wo2.opt()],
            )

            # Consts go through the Scalar engine's HWDGE queue so the Sync
            # queue is a pure x stream; Bv is broadcast on-chip (saves 2MB
            # of HBM reads vs a host-tiled [128, DIN] input).
            bv_sb = cst.tile([128, DIN], F32, tag="bv")
            nc.scalar.dma_start(bv_sb[:], bv1.broadcast_to([128, DIN]))
            a2_sb = cst.tile([128, DOUT], F32, tag="a2")
            nc.scalar.dma_start(a2_sb[:], a2b[:])
            ga_sb = cst.tile([128, DOUT], F32, tag="ga")
            nc.scalar.dma_start(ga_sb[:], gab[:])
            if with_beta:
                be_sb = cst.tile([128, DOUT], F32, tag="be")
                nc.scalar.dma_start(be_sb[:], beb[:])

            t_all = cst.tile([128, M_TILES], F32, tag="t")

            # Phase 1: t[:, m] = rowwise dot(x_tile, Bv)
            # DVE does the elementwise product; ACT reduces via Copy+accum.
            # The last tile is split into 4 chunks so its mult+reduce
            # pipelines right behind the final DMA instead of serializing
            # 8us of DVE+ACT after it.
            t_parts = cst.tile([128, 4], F32, tag="tparts")
            for m in range(M_TILES):
                x_sb = xp.tile([128, DIN], F32, tag="x")
                nc.sync.dma_start(x_sb[:], xs[m * 128 : (m + 1) * 128, :])
                scr = scrp.tile([128, DIN], F32, tag="scr")
                scr3 = scrp.tile([128, DIN], F32, tag="scr3")
                if m < M_TILES - 1:
                    nc.vector.tensor_mul(scr[:], x_sb[:], bv_sb[:])
                    nc.scalar.activation(
                        scr3[:],
                        scr[:],
                        mybir.ActivationFunctionType.Copy,
                        accum_out=t_all[:, m : m + 1],
                    )
                else:
                    q = DIN // 4
                    for c in range(4):
                        sl = slice(c * q, (c + 1) * q)
                        nc.vector.tensor_mul(scr[:, sl], x_sb[:, sl], bv_sb[:, sl])
                        nc.scalar.activation(
                            scr3[:, sl],
                            scr[:, sl],
                            mybir.ActivationFunctionType.Copy,
                            accum_out=t_parts[:, c : c + 1],
                        )
                    nc.vector.tensor_reduce(
                        out=t_all[:, M_TILES - 1 : M_TILES],
                        in_=t_parts[:],
                        axis=mybir.AxisListType.X,
                        op=mybir.AluOpType.add,
                    )

            # Phase 2: local partial sums -> cross-partition reduce -> AllReduce
            sp = cst.tile([128, 2], F32, tag="sp")
            nc.vector.tensor_reduce(
                out=sp[:, 0:1],
                in_=t_all[:],
                axis=mybir.AxisListType.X,
                op=mybir.AluOpType.add,
            )
            scr2 = cst.tile([128, M_TILES], F32, tag="scr2")
            nc.scalar.activation(
                scr2[:],
                t_all[:],
                mybir.ActivationFunctionType.Square,
                accum_out=sp[:, 1:2],
            )
            ones_c = cst.tile([128, 1], F32, tag="ones")
            nc.vector.memset(ones_c[:], 1.0)
            s_ps = ps.tile([2, 1], F32, tag="sps")
            nc.tensor.matmul(s_ps[:], sp[:], ones_c[:], start=True, stop=True)

            s_sb = cst.tile([2, 1], F32, tag="ssb")
            nc.vector.tensor_copy(s_sb[:], s_ps[:])
            bi = dram.tile([2, 1], F32, tag="bi")
            bo = dram.tile([2 * N_CORES, 1], F32, tag="bo")
            nc.sync.dma_start(bi[:], s_sb[:])
            nc.gpsimd.collective_compute(
                "AllGather",
                mybir.AluOpType.bypass,
                replica_groups=[list(range(N_CORES))],
                ins=[bi.opt()],
                outs=[bo.opt()],
            )
            # bo holds [s1_r0, s2_r0, s1_r1, s2_r1, ...]; broadcast to all
            # partitions then reduce over ranks with a stride-2 view.
            sb16 = cst.tile([128, 2 * N_CORES], F32, tag="sb16")
            nc.sync.dma_start(
                sb16[:],
                bo.rearrange("a b -> b a").broadcast_to([128, 2 * N_CORES]),
            )
            sb2 = cst.tile([128, 2], F32, tag="sb2")
            nc.vector.tensor_reduce(
                out=sb2[:],
                in_=sb16.rearrange("p (r s) -> p s r", s=2),
                axis=mybir.AxisListType.X,
                op=mybir.AluOpType.add,
            )

            # Stats math (replicated on all 128 partitions)
            mcol = cst.tile([128, 1], F32, tag="mcol")
            nc.vector.tensor_scalar_mul(mcol[:], sb2[:, 0:1], 1.0 / B_FULL)
            ecol = cst.tile([128, 1], F32, tag="ecol")
            nc.vector.tensor_scalar_mul(ecol[:], sb2[:, 1:2], 1.0 / B_FULL)
            msq = cst.tile([128, 1], F32, tag="msq")
            nc.vector.tensor_mul(msq[:], mcol[:], mcol[:])
            vcol = cst.tile([128, 1], F32, tag="vcol")
            nc.vector.tensor_sub(vcol[:], ecol[:], msq[:])

            v1 = cst.tile([128, DOUT], F32, tag="v1")
            nc.vector.tensor_scalar(
                v1[:],
                a2_sb[:],
                vcol[:, 0:1],
                BN_EPS,
                op0=mybir.AluOpType.mult,
                op1=mybir.AluOpType.add,
            )
            v3 = cst.tile([128, DOUT], F32, tag="v3")
            nc.scalar.activation(
                v3[:], v1[:], mybir.ActivationFunctionType.Abs_reciprocal_sqrt
            )
            u_b = cst.tile([128, DOUT], F32, tag="ub")
            nc.vector.tensor_mul(u_b[:], v3[:], ga_sb[:])

            tcall = cst.tile([128, M_TILES], F32, tag="tc")
            nc.vector.tensor_scalar_sub(tcall[:], t_all[:], mcol[:, 0:1])

            # Phase 3: act = lrelu(u_b * tc[m] (+ beta))
            for m in range(M_TILES):
                o_sb = op.tile([128, DOUT], F32, tag="o")
                if with_beta:
                    y = op.tile([128, DOUT], F32, tag="y")
                    nc.vector.tensor_scalar_mul(y[:], u_b[:], tcall[:, m : m + 1])
                    nc.vector.tensor_add(y[:], y[:], be_sb[:])
                    nc.scalar.activation(
                        o_sb[:], y[:], _act_func(), alpha=NEG_SLOPE
                    )
                elif m % 2 == 0:
                    nc.scalar.activation(
                        o_sb[:],
                        u_b[:],
                        _act_func(),
                        scale=tcall[:, m : m + 1],
                        alpha=NEG_SLOPE,
                    )
                else:
                    # DVE leaky-relu: y = u*tc; out = max(y, 0.2*y)
                    y = op.tile([128, DOUT], F32, tag="y2")
                    z = op.tile([128, DOUT], F32, tag="z2")
                    nc.vector.tensor_scalar_mul(y[:], u_b[:], tcall[:, m : m + 1])
                    nc.vector.tensor_scalar_mul(z[:], y[:], NEG_SLOPE)
                    nc.vector.tensor_tensor(
                        o_sb[:], y[:], z[:], op=mybir.AluOpType.max
                    )
                dma_eng = nc.sync if m % 2 == 0 else nc.scalar
                dma_eng.dma_start(out[m * 128 : (m + 1) * 128, :], o_sb[:])

    nc.compile()
    return nc


def _get_nc(with_beta: bool):
    if with_beta not in _CACHE:
        _CACHE[with_beta] = _build(with_beta)
    return _CACHE[with_beta]


def kernel(x, alphas_A, controls_A, controls_B, linear_bias, bn_gamma, bn_beta,
           _trace=False):
    x = np.asarray(x, dtype=np.float32)
    alphas_A = np.asarray(alphas_A, dtype=np.float32)
    controls_A = np.asarray(controls_A, dtype=np.float32)
    controls_B = np.asarray(controls_B, dtype=np.float32)
    bn_gamma = np.asarray(bn_gamma, dtype=np.float32)
    bn_beta = np.asarray(bn_beta, dtype=np.float32)

    A = (alphas_A * controls_A).sum(axis=0).astype(np.float32)          # (256,)
    Bv = (controls_B * alphas_A.T).sum(axis=1).astype(np.float32)       # (4096,)

    bv1 = np.ascontiguousarray(Bv.reshape(1, DIN))
    a2b = np.ascontiguousarray(np.broadcast_to(A * A, (128, DOUT)))
    gab = np.ascontiguousarray(np.broadcast_to(bn_gamma * A, (128, DOUT)))
    with_beta = bool(np.any(bn_beta != 0))

    nc = _get_nc(with_beta)
    in_maps = []
    for c in range(N_CORES):
        im = {
            "xs": np.ascontiguousarray(x[c * B_SHARD : (c + 1) * B_SHARD]),
            "bv1": bv1,
            "a2b": a2b,
            "gab": gab,
        }
        if with_beta:
            im["beb"] = np.ascontiguousarray(
                np.broadcast_to(bn_beta, (128, DOUT)))
        in_maps.append(im)

    res = run_bass_kernel_spmd(
        nc, in_maps, core_ids=list(range(N_CORES)), trace=_trace
    )
    out = np.concatenate([r["out"] for r in res.results], axis=0)
    if _trace:
        return out, res
    return out

